# revision 1
# baseline (speedup 1.0000x reference)
"""Trainium2 Bass kernel for nn_CrossLocal (cross-attention + convs + BN +
bilinear resizes), distributed over 8 NeuronCores.

Sharding: launch 1 (attention) is data-parallel over (batch, query-half) --
core = 2*b + qhalf owns 2048 of the 4096 query positions of batch b.
BatchNorm statistics are combined on the host (tiny: 64 channels x 2), then
launch 2 (BN + 2x bilinear upsample + residual) is data-parallel over
(batch, image-row-half).

Algebra used on device (softmax-invariant rewrite):
  S[q,m] = g(cr)[:,q] . theta(cr)[:,m]
         = cr[:,q]^T A cr[:,m] + u.cr[:,m] + (terms constant in m -> dropped)
  with A = g_w^T theta_w, u = theta_w^T g_b.  We compute w = A @ cr once,
  then S_T[m-tile] = w[:,mt]^T @ crq (K=64, row-tiled in pairs on the PE),
  exp via ScalarE with the per-partition bias r[m] = u^T cr folded in, and
  attn @ z via z^T tiles carrying an extra ones column that accumulates the
  softmax denominator for free.  z^T = (phi(avgpool2x2(main)))^T is produced
  directly in transposed layout by matmul-ing strided views of main against
  phi^T (the 2x2-pool is folded into the contraction; 0.25 folded into the
  weights).  Conv-W bias is folded via an augmented lhsT whose extra row
  multiplies the denominator column.  Big matmuls run in float32r (TF32).
"""

import os
from contextlib import ExitStack

import numpy as np

import concourse.bass as bass
import concourse.tile as tile
from concourse import bacc, mybir
from concourse.bass_utils import run_bass_kernel_spmd

f32 = mybir.dt.float32
f32r = mybir.dt.float32r
AF = mybir.ActivationFunctionType
ALU = mybir.AluOpType

B, C, CI = 4, 64, 32
HM, HC = 128, 64
NC = HC * HC          # 4096 keys / cross positions
NQ = NC // 2          # 2048 queries per core
QB = 1024             # query block (psum-friendly)
NMT = NC // 128       # 32 m-tiles of 128 keys
BN_EPS = 1e-5
CORES = list(range(8))

_cache = {}
last_profile = {}  # launch name -> BassKernelResults (when BASS_PROFILE=1)


def _run(nc, in_maps, name):
    trace = os.environ.get("BASS_PROFILE", "") == "1"
    tmpdir = None
    if trace:
        tmpdir = os.path.join("/tmp/bass_traces", name)
        os.makedirs(tmpdir, exist_ok=True)
    br = run_bass_kernel_spmd(
        nc, in_maps, core_ids=CORES, trace=trace, tmpdir=tmpdir
    )
    if trace:
        last_profile[name] = br
    return br.results


def _build_l1():
    nc = bacc.Bacc("TRN2", target_bir_lowering=False, debug=False)
    d_cross = nc.dram_tensor("cross", [C, NC], f32, kind="ExternalInput").ap()
    d_crossq = nc.dram_tensor("crossq", [C, NQ], f32, kind="ExternalInput").ap()
    d_main = nc.dram_tensor("main", [C, HM * HM], f32, kind="ExternalInput").ap()
    d_amat = nc.dram_tensor("amat", [C, 128], f32, kind="ExternalInput").ap()
    d_uvec = nc.dram_tensor("uvec", [C, 1], f32, kind="ExternalInput").ap()
    d_phiw = nc.dram_tensor("phiw", [128, CI], f32, kind="ExternalInput").ap()
    d_phib = nc.dram_tensor("phib", [1, CI + 1], f32, kind="ExternalInput").ap()
    d_wwt = nc.dram_tensor("wwt", [CI + 1, C], f32, kind="ExternalInput").ap()
    d_ones = nc.dram_tensor("ones", [1, 128], f32, kind="ExternalInput").ap()
    d_h = nc.dram_tensor("h_out", [C, NQ], f32, kind="ExternalOutput").ap()
    d_st = nc.dram_tensor("stats_out", [C, 2], f32, kind="ExternalOutput").ap()

    with ExitStack() as ctx:
        tc = ctx.enter_context(tile.TileContext(nc))
        const = ctx.enter_context(tc.tile_pool(name="const", bufs=1))
        big = ctx.enter_context(tc.tile_pool(name="big", bufs=1))
        expp = ctx.enter_context(tc.tile_pool(name="expp", bufs=12))
        small = ctx.enter_context(tc.tile_pool(name="small", bufs=2))
        ps_s = ctx.enter_context(tc.tile_pool(name="ps_s", bufs=2, space="PSUM"))
        ps_num = ctx.enter_context(tc.tile_pool(name="ps_num", bufs=1, space="PSUM"))
        ps_aux = ctx.enter_context(tc.tile_pool(name="ps_aux", bufs=2, space="PSUM"))

        # --- warm the exp table ASAP (one-time ~2.7us load) ---
        warm = const.tile([1, 1], f32, tag="warm")
        nc.vector.memset(warm, 0.0)
        nc.scalar.activation(out=warm, in_=warm, func=AF.Exp)

        # --- constant loads ---
        am = const.tile([C, 128], f32r, tag="am")
        nc.gpsimd.dma_start(out=am, in_=d_amat)
        uv = const.tile([C, 1], f32, tag="uv")
        nc.sync.dma_start(out=uv, in_=d_uvec)
        phiw = const.tile([128, CI], f32, tag="phiw")
        nc.sync.dma_start(out=phiw, in_=d_phiw)
        phib = const.tile([1, CI + 1], f32, tag="phib")
        nc.sync.dma_start(out=phib, in_=d_phib)
        wwt = const.tile([CI + 1, C], f32, tag="wwt")
        nc.sync.dma_start(out=wwt, in_=d_wwt)
        ones = const.tile([1, 128], f32, tag="ones")
        nc.sync.dma_start(out=ones, in_=d_ones)

        # --- cross loads (cast to f32r) ---
        cr = big.tile([C, NC], f32r, tag="cr")
        nc.gpsimd.dma_start(out=cr, in_=d_cross)
        crq = big.tile([128, NQ], f32r, tag="crq")
        nc.gpsimd.dma_start(out=crq[0:C, :], in_=d_crossq)
        nc.gpsimd.dma_start(out=crq[C:128, :], in_=d_crossq)

        # --- w = A @ cr, duplicated on both partition halves via amat's
        # duplicated M columns; r = u^T cr as per-partition bias tiles ---
        w_dup = big.tile([128, NC], f32r, tag="w_dup")
        for k in range(NC // 512):
            pw = ps_aux.tile([128, 512], f32, tag="aux")
            nc.tensor.matmul(
                out=pw, lhsT=am, rhs=cr[:, k * 512:(k + 1) * 512],
                start=True, stop=True,
            )
            nc.vector.tensor_copy(out=w_dup[:, k * 512:(k + 1) * 512], in_=pw)

        cr32 = big.tile([C, NC], f32, tag="cr32")
        nc.sync.dma_start(out=cr32, in_=d_cross)
        pr = ps_aux.tile([128, 512], f32, tag="aux")
        for t in range(NMT):
            nc.tensor.matmul(
                out=pr[:, t:t + 1], lhsT=cr32[:, t * 128:(t + 1) * 128], rhs=uv,
                start=True, stop=True,
            )
        r_sb = const.tile([128, NMT], f32, tag="r_sb")
        nc.vector.tensor_copy(out=r_sb, in_=pr[:, 0:NMT])

        # --- main2: h-parity-stacked main rows for K=128 pooled-phi matmuls
        # main2[c,    i*128+w] = main[c, (2i  )*128+w]
        # main2[c+64, i*128+w] = main[c, (2i+1)*128+w]
        mv = d_main.rearrange("c (i di w) -> c i di w", di=2, w=HM)
        main2 = big.tile([128, (HM // 2) * HM], f32, tag="main2")
        m2v = main2.rearrange("p (i w) -> p i w", w=HM)
        NCH = 8  # chunks per parity to spread DMA queues
        for cki in range(NCH):
            lo, hi = cki * (HM // 2) // NCH, (cki + 1) * (HM // 2) // NCH
            nc.sync.dma_start(out=m2v[0:C, lo:hi, :], in_=mv[:, lo:hi, 0, :])
            nc.sync.dma_start(out=m2v[C:128, lo:hi, :], in_=mv[:, lo:hi, 1, :])

        # --- z_aug[:, t, :]: [128 m, 33] = [z^T | denominator-ones column]
        # zT[m,ci] = sum_c 0.25*(4-point pool)(main)[c,m] * phi_w[ci,c] + phi_b
        # expressed as 2 K=128 matmuls over strided main2 views (dj=0,1)
        # plus a K=1 ones matmul adding [phi_b, 1.0].
        z_aug = big.tile([128, NMT, CI + 1], f32r, tag="z_aug")
        for t in range(NMT):
            pz = ps_aux.tile([128, 512], f32, tag="aux")
            for dj in range(2):
                # [128, 2, 64] -> M=128: m = 64*isub + j, value
                # main2[p, (2t+isub)*128 + 2j+dj]
                lhs = m2v[:, 2 * t:2 * t + 2, dj::2]
                nc.tensor.matmul(
                    out=pz[:, 1:CI + 1], lhsT=lhs, rhs=phiw,
                    start=(dj == 0), stop=False,
                )
            nc.tensor.matmul(
                out=pz[:, 0:CI + 1], lhsT=ones, rhs=phib,
                start=False, stop=True, skip_group_check=True,
            )
            nc.vector.tensor_copy(out=z_aug[:, t, :], in_=pz[:, 0:CI + 1])

        # --- attention main loop ---
        h_sb = big.tile([C, NQ], f32, tag="h_sb")
        for qb in range(NQ // QB):
            q0 = qb * QB
            num = ps_num.tile([128, QB], f32, tag="num")
            for t in range(NMT):
                half = 64 * (t % 2)
                s_ps = ps_s.tile([128, QB], f32, tag="s")
                for k in range(QB // 512):
                    nc.tensor.matmul(
                        out=s_ps[:, k * 512:(k + 1) * 512],
                        lhsT=w_dup[half:half + C, t * 128:(t + 1) * 128],
                        rhs=crq[half:half + C, q0 + k * 512:q0 + (k + 1) * 512],
                        start=True, stop=True,
                    )
                es = expp.tile([128, QB], f32r, tag="es")
                nc.scalar.activation(
                    out=es, in_=s_ps, func=AF.Exp,
                    bias=r_sb[:, t:t + 1], scale=1.0,
                )
                for k in range(QB // 512):
                    nc.tensor.matmul(
                        out=num[0:CI + 1, k * 512:(k + 1) * 512],
                        lhsT=z_aug[:, t, :],
                        rhs=es[:, k * 512:(k + 1) * 512],
                        start=(t == 0), stop=(t == NMT - 1),
                        skip_group_check=True,
                    )
            # epilogue for this query block
            nsb = small.tile([CI + 1, QB], f32, tag="nsb")
            nc.vector.tensor_copy(out=nsb, in_=num[0:CI + 1, :])
            for k in range(QB // 512):
                sl = slice(k * 512, (k + 1) * 512)
                dbc = ps_aux.tile([128, 512], f32, tag="aux")
                nc.tensor.matmul(
                    out=dbc[0:C, :], lhsT=ones[:, 0:C], rhs=nsb[0:1, sl],
                    start=True, stop=True,
                )
                rec = small.tile([C, 512], f32, tag="rec")
                nc.vector.reciprocal(out=rec, in_=dbc[0:C, :])
                hp = ps_aux.tile([128, 512], f32, tag="aux")
                nc.tensor.matmul(
                    out=hp[0:C, :], lhsT=wwt, rhs=nsb[:, sl],
                    start=True, stop=True,
                )
                nc.vector.tensor_tensor(
                    out=h_sb[:, q0 + k * 512:q0 + (k + 1) * 512],
                    in0=hp[0:C, :], in1=rec, op=ALU.mult,
                )

        # --- BN partial stats + stores ---
        st = small.tile([C, 2], f32, tag="st")
        nc.vector.tensor_reduce(
            out=st[:, 0:1], in_=h_sb, axis=mybir.AxisListType.X, op=ALU.add
        )
        sq = big.tile([C, NQ], f32, tag="sq")
        nc.scalar.activation(
            out=sq, in_=h_sb, func=AF.Square, accum_out=st[:, 1:2]
        )
        nc.sync.dma_start(out=d_h, in_=h_sb)
        nc.sync.dma_start(out=d_st, in_=st)

    nc.compile()
    return nc


def _build_l2():
    nc = bacc.Bacc("TRN2", target_bir_lowering=False, debug=False)
    d_hs = nc.dram_tensor("hs", [C, 34 * HC], f32, kind="ExternalInput").ap()
    d_mn = nc.dram_tensor("mainr", [C, 64 * HM], f32, kind="ExternalInput").ap()
    d_a = nc.dram_tensor("abn", [C, 1], f32, kind="ExternalInput").ap()
    d_b = nc.dram_tensor("bbn", [C, 1], f32, kind="ExternalInput").ap()
    d_o = nc.dram_tensor("outp", [C, 64 * HM], f32, kind="ExternalOutput").ap()

    with ExitStack() as ctx:
        tc = ctx.enter_context(tile.TileContext(nc))
        sb = ctx.enter_context(tc.tile_pool(name="sb", bufs=1))
        cst = ctx.enter_context(tc.tile_pool(name="cst", bufs=1))

        hs = sb.tile([C, 34, HC], f32, tag="hs")
        nc.sync.dma_start(out=hs, in_=d_hs.rearrange("c (r w) -> c r w", w=HC))
        mn = sb.tile([C, 64 * HM], f32, tag="mn")
        nc.sync.dma_start(out=mn, in_=d_mn)
        ab = cst.tile([C, 1], f32, tag="ab")
        nc.sync.dma_start(out=ab, in_=d_a)
        bb = cst.tile([C, 1], f32, tag="bb")
        nc.sync.dma_start(out=bb, in_=d_b)

        hbn = sb.tile([C, 34, HC], f32, tag="hbn")
        nc.vector.tensor_scalar(
            out=hbn, in0=hs, scalar1=ab, scalar2=bb, op0=ALU.mult, op1=ALU.add
        )
        # row upsample (even/odd stencils), stored 4x true, cols padded by 1
        rp = sb.tile([C, 64, HC + 2], f32, tag="rp")
        nc.vector.scalar_tensor_tensor(
            out=rp[:, 0::2, 1:HC + 1], in0=hbn[:, 1:33, :], scalar=3.0,
            in1=hbn[:, 0:32, :], op0=ALU.mult, op1=ALU.add,
        )
        nc.vector.scalar_tensor_tensor(
            out=rp[:, 1::2, 1:HC + 1], in0=hbn[:, 1:33, :], scalar=3.0,
            in1=hbn[:, 2:34, :], op0=ALU.mult, op1=ALU.add,
        )
        nc.vector.tensor_copy(out=rp[:, :, 0:1], in_=rp[:, :, 1:2])
        nc.vector.tensor_copy(out=rp[:, :, HC + 1:HC + 2], in_=rp[:, :, HC:HC + 1])
        # col upsample
        up = sb.tile([C, 64, HM], f32, tag="up")
        nc.vector.scalar_tensor_tensor(
            out=up[:, :, 0::2], in0=rp[:, :, 1:HC + 1], scalar=3.0,
            in1=rp[:, :, 0:HC], op0=ALU.mult, op1=ALU.add,
        )
        nc.vector.scalar_tensor_tensor(
            out=up[:, :, 1::2], in0=rp[:, :, 1:HC + 1], scalar=3.0,
            in1=rp[:, :, 2:HC + 2], op0=ALU.mult, op1=ALU.add,
        )
        out_sb = sb.tile([C, 64 * HM], f32, tag="out_sb")
        nc.vector.tensor_tensor(
            out=out_sb, in0=up.rearrange("c r w -> c (r w)"), in1=mn, op=ALU.add
        )
        nc.sync.dma_start(out=d_o, in_=out_sb)

    nc.compile()
    return nc


def kernel(main_feature, cross_feature, g_w, g_b, theta_w, theta_b,
           phi_w, phi_b, w_w, w_b, bn_gamma, bn_beta):
    main_feature = np.ascontiguousarray(main_feature, dtype=np.float32)
    cross_feature = np.ascontiguousarray(cross_feature, dtype=np.float32)

    if "l1" not in _cache:
        _cache["l1"] = _build_l1()
    if "l2" not in _cache:
        _cache["l2"] = _build_l2()

    # host weight prep (tiny, fp64 for accuracy)
    g_w64, th_w64 = g_w.astype(np.float64), theta_w.astype(np.float64)
    A_T = (th_w64.T @ g_w64).astype(np.float32)          # lhsT[c',c] = A[c,c']
    amat = np.concatenate([A_T, A_T], axis=1)            # [64, 128] dup M
    uvec = (th_w64.T @ g_b.astype(np.float64)).astype(np.float32)[:, None]
    phiw = np.concatenate([0.25 * phi_w.T, 0.25 * phi_w.T], axis=0)  # [128,32]
    phib = np.concatenate([[1.0], phi_b]).astype(np.float32)[None, :]  # [1,33]
    wwt = np.concatenate([w_b[None, :], w_w.T], axis=0).astype(np.float32)
    ones = np.ones((1, 128), dtype=np.float32)

    in_maps = []
    for core in CORES:
        b, qh = core // 2, core % 2
        cr = cross_feature[b].reshape(C, NC)
        in_maps.append({
            "cross": cr,
            "crossq": np.ascontiguousarray(cr[:, qh * NQ:(qh + 1) * NQ]),
            "main": main_feature[b].reshape(C, HM * HM),
            "amat": amat, "uvec": uvec, "phiw": phiw, "phib": phib,
            "wwt": wwt, "ones": ones,
        })
    res1 = _run(_cache["l1"], in_maps, "l1")

    # gather h, combine BN stats
    h = np.empty((B, C, NC), dtype=np.float32)
    s_sum = np.zeros(C, dtype=np.float64)
    s_sq = np.zeros(C, dtype=np.float64)
    for core in CORES:
        b, qh = core // 2, core % 2
        h[b][:, qh * NQ:(qh + 1) * NQ] = res1[core]["h_out"]
        s_sum += res1[core]["stats_out"][:, 0].astype(np.float64)
        s_sq += res1[core]["stats_out"][:, 1].astype(np.float64)
    n = float(B * NC)
    mean = s_sum / n
    var = s_sq / n - mean * mean
    scale = bn_gamma.astype(np.float64) / np.sqrt(var + BN_EPS)
    shift = bn_beta.astype(np.float64) - mean * scale
    abn = (scale / 16.0).astype(np.float32)[:, None]
    bbn = (shift / 16.0).astype(np.float32)[:, None]

    hg = h.reshape(B, C, HC, HC)
    in_maps2 = []
    for core in CORES:
        b, half = core // 2, core % 2
        idx = np.clip(np.arange(-1, 33) + 32 * half, 0, HC - 1)
        in_maps2.append({
            "hs": np.ascontiguousarray(hg[b][:, idx, :]).reshape(C, 34 * HC),
            "mainr": np.ascontiguousarray(
                main_feature[b][:, 64 * half:64 * half + 64, :]
            ).reshape(C, 64 * HM),
            "abn": abn, "bbn": bbn,
        })
    res2 = _run(_cache["l2"], in_maps2, "l2")

    out = np.empty((B, C, HM, HM), dtype=np.float32)
    for core in CORES:
        b, half = core // 2, core % 2
        out[b][:, 64 * half:64 * half + 64, :] = (
            res2[core]["outp"].reshape(C, 64, HM)
        )
    return out

